# revision 7
# baseline (speedup 1.0000x reference)
"""Multi-head attention (B=2, S=2048, D=1024, H=16 heads, causal) on 8 trn2 cores.

Sharding: heads across cores (2 heads = 128 channels per core).
  - W_q/W_k/W_v column-sharded: each core projects all tokens to its 128 channels.
  - Attention per (batch, head) fully local to a core.
  - W_o row-sharded: each core computes a partial output projection; partials
    are summed on the host (the unshard step), then b_o is added.

Device layout: everything transposed (channels on partitions, tokens on free).
  - Scores computed as S^T blocks [128 k-tok, 512 q-tok] so exp is elementwise
    and the softmax sum comes for free from a ones-column appended to V.
  - Causal structure: host inspects the mask, emits only non-empty blocks and
    multiplies boundary blocks by 0/1 pattern tiles.

All matmuls run in bf16 (inputs cast on host) with fp32 PSUM accumulation;
the partial output is returned bf16 and reduced in fp32 on the host.
"""

import sys

import numpy as np

try:
    import concourse.bass as bass  # noqa: F401
except ImportError:  # pragma: no cover
    sys.path.insert(0, "/opt/trn_rl_repo")

import ml_dtypes

import concourse.mybir as mybir
import concourse.tile as tile
from concourse import bacc, bass_utils
from concourse.masks import make_identity

P = 128
B, S, D = 2, 2048, 1024
H, DK = 16, 64
N_CORES = 8
HPC = H // N_CORES  # heads per core = 2
CH = HPC * DK  # channels per core = 128
TOK = B * S  # 4096
NKB = S // P  # k-blocks per batch = 16
CW = 512  # q column width
NJ = S // CW  # q columns per batch = 4
NTG = S // CW  # 512-token projection groups per batch = 4
XC = D // P  # x-dim chunks = 8
MO = D // P  # output-channel chunks = 8

BF16 = mybir.dt.bfloat16
F32 = mybir.dt.float32
NPBF16 = ml_dtypes.bfloat16

_BUILD_CACHE = {}


def _analyze_mask(mask):
    """Per q-column block plan from the (1,1,S,S) boolean mask.

    Returns (plan, pats): plan[j] = tuple of (k_block, pattern_id) with
    pattern_id = -1 for fully-valid blocks; pats is a (P, U, CW) float array of
    0/1 tiles in [k, q] layout (U >= 1).
    """
    m = np.asarray(mask).reshape(S, S).astype(bool)  # m[q, k]
    pat_index = {}
    pats = []
    plan = []
    for j in range(NJ):
        q0 = j * CW
        blocks = []
        for bk in range(NKB):
            sub = m[q0 : q0 + CW, bk * P : (bk + 1) * P]  # (CW q, P k)
            if not sub.any():
                continue
            if sub.all():
                blocks.append((bk, -1))
                continue
            tile_kq = np.ascontiguousarray(sub.T).astype(np.float32)  # (P k, CW q)
            key = tile_kq.tobytes()
            if key not in pat_index:
                pat_index[key] = len(pats)
                pats.append(tile_kq)
            blocks.append((bk, pat_index[key]))
        plan.append(tuple(blocks))
    if not pats:
        pats.append(np.ones((P, CW), np.float32))
    pat_arr = np.stack(pats, axis=1)  # (P, U, CW)
    return tuple(plan), pat_arr


def _build(plan, n_pat):
    nc = bacc.Bacc(
        "TRN2",
        target_bir_lowering=False,
        debug=False,
        enable_asserts=True,
        num_devices=N_CORES,
    )
    xq = nc.dram_tensor("xq", [D, TOK], BF16, kind="ExternalInput").ap()
    xk = nc.dram_tensor("xk", [D, TOK], BF16, kind="ExternalInput").ap()
    xv = nc.dram_tensor("xv", [D, TOK], BF16, kind="ExternalInput").ap()
    wq = nc.dram_tensor("wq", [D, CH], BF16, kind="ExternalInput").ap()
    wk = nc.dram_tensor("wk", [D, CH], BF16, kind="ExternalInput").ap()
    wv = nc.dram_tensor("wv", [D, CH], BF16, kind="ExternalInput").ap()
    wo = nc.dram_tensor("wo", [CH, D], BF16, kind="ExternalInput").ap()
    bq = nc.dram_tensor("bq", [CH, 1], F32, kind="ExternalInput").ap()
    bk_ = nc.dram_tensor("bk", [CH, 1], F32, kind="ExternalInput").ap()
    bv = nc.dram_tensor("bv", [CH, 1], F32, kind="ExternalInput").ap()
    mpat = nc.dram_tensor("mpat", [P, n_pat, CW], BF16, kind="ExternalInput").ap()
    out = nc.dram_tensor("out", [D, TOK], BF16, kind="ExternalOutput").ap()

    out_r = out.rearrange("(mo p) t -> p mo t", p=P)

    with tile.TileContext(nc) as tc:
        with (
            tc.tile_pool(name="const", bufs=1) as const,
            tc.tile_pool(name="persist", bufs=1) as persist,
            tc.tile_pool(name="xt", bufs=3) as xtp,
            tc.tile_pool(name="a2", bufs=3) as a2p,
            tc.tile_pool(name="yt", bufs=4) as ytp,
            tc.tile_pool(name="ob", bufs=3) as obp,
            tc.tile_pool(name="small", bufs=3) as small,
            tc.tile_pool(name="pp", bufs=2, space="PSUM") as pp,
            tc.tile_pool(name="s2", bufs=2, space="PSUM") as s2p,
            tc.tile_pool(name="op", bufs=2, space="PSUM") as opp,
        ):
            ident = const.tile([P, P], BF16, tag="ident")
            make_identity(nc, ident)
            ones_sb = const.tile([1, DK], F32, tag="ones")
            nc.gpsimd.memset(ones_sb[:], 1.0)

            w_sb = {}
            b_sb = {}
            for name, wdram, bdram in (
                ("q", wq, bq),
                ("k", wk, bk_),
                ("v", wv, bv),
            ):
                w_sb[name] = const.tile([P, XC, CH], BF16, tag=f"w{name}", name=f"w{name}")
                nc.sync.dma_start(
                    w_sb[name][:], wdram.rearrange("(o p) c -> p o c", p=P)
                )
                b_sb[name] = const.tile([CH, 1], F32, tag=f"b{name}", name=f"b{name}")
                nc.sync.dma_start(b_sb[name][:], bdram)
            wo_sb = const.tile([CH, D], BF16, tag="wo")
            nc.sync.dma_start(wo_sb[:], wo)
            mask_sb = const.tile([P, n_pat, CW], BF16, tag="mpat")
            nc.sync.dma_start(mask_sb[:], mpat)

            # V with a trailing ones column, per (batch, local head): [k, d+1]
            vaug = {}
            for b in range(B):
                for hl in range(HPC):
                    t = persist.tile([P, NKB, DK + 1], BF16, tag=f"vaug{b}{hl}", name=f"vaug{b}{hl}")
                    nc.gpsimd.memset(t[:, :, DK : DK + 1], 1.0)
                    vaug[b, hl] = t

            qt, kt, vt = {}, {}, {}
            for b in range(B):
                # --- projections: channels on partitions, tokens on free ---
                for name, xdram, dst in (("k", xk, kt), ("q", xq, qt), ("v", xv, vt)):
                    dst[b] = persist.tile([CH, S], BF16, tag=f"{name}t{b}", name=f"{name}t{b}")
                    xr = xdram.rearrange("(o p) t -> p o t", p=P)
                    for tg in range(NTG):
                        t0 = b * S + tg * CW
                        xt = xtp.tile([P, XC, CW], BF16, tag="xt")
                        nc.sync.dma_start(xt[:], xr[:, :, t0 : t0 + CW])
                        ps = pp.tile([CH, CW], F32, tag="pp")
                        for xc in range(XC):
                            nc.tensor.matmul(
                                ps[:],
                                lhsT=w_sb[name][:, xc, :],
                                rhs=xt[:, xc, :],
                                start=(xc == 0),
                                stop=(xc == XC - 1),
                            )
                        nc.vector.tensor_add(
                            dst[b][:, tg * CW : (tg + 1) * CW],
                            ps[:],
                            b_sb[name][:, 0:1].to_broadcast((CH, CW)),
                        )

                # --- V transpose into vaug (both heads per 128x128 tile) ---
                for kb in range(NKB):
                    tp = pp.tile([P, P], BF16, tag="pp")
                    nc.tensor.transpose(tp[:], vt[b][:, kb * P : (kb + 1) * P], ident[:])
                    for hl in range(HPC):
                        nc.vector.tensor_copy(
                            vaug[b, hl][:, kb, 0:DK], tp[:, hl * DK : (hl + 1) * DK]
                        )

                # --- attention + output projection, per q column ---
                for j in range(NJ):
                    blocks = plan[j]
                    q0 = j * CW
                    yt = ytp.tile([CH, CW], BF16, tag="yt")
                    if not blocks:
                        nc.gpsimd.memset(yt[:], 0.0)
                    else:
                        ops = {}
                        for hl in range(HPC):
                            ops[hl] = opp.tile([DK + 1, CW], F32, tag="op", name=f"op{hl}")
                        nblk = len(blocks)
                        for i, (bk, pid) in enumerate(blocks):
                            k0 = bk * P
                            s2 = s2p.tile([P, HPC, CW], F32, tag="s2")
                            for hl in range(HPC):
                                hs = slice(hl * DK, (hl + 1) * DK)
                                nc.tensor.matmul(
                                    s2[:, hl, :],
                                    lhsT=kt[b][hs, k0 : k0 + P],
                                    rhs=qt[b][hs, q0 : q0 + CW],
                                    start=True,
                                    stop=True,
                                )
                            a2 = a2p.tile([P, HPC, CW], BF16, tag="a2")
                            nc.scalar.activation(
                                a2[:], s2[:], mybir.ActivationFunctionType.Exp,
                                scale=0.125,
                            )
                            if pid >= 0:
                                nc.vector.tensor_tensor(
                                    a2[:],
                                    a2[:],
                                    mask_sb[:, pid : pid + 1, :].to_broadcast(
                                        (P, HPC, CW)
                                    ),
                                    mybir.AluOpType.mult,
                                )
                            for hl in range(HPC):
                                nc.tensor.matmul(
                                    ops[hl][:],
                                    lhsT=vaug[b, hl][:, bk, :],
                                    rhs=a2[:, hl, :],
                                    start=(i == 0),
                                    stop=(i == nblk - 1),
                                )
                        # normalize: rows 0:DK divided by the sums row DK
                        for hl in range(HPC):
                            sums = small.tile([1, CW], F32, tag="sums")
                            nc.vector.tensor_copy(sums[:], ops[hl][DK : DK + 1, :])
                            rec = small.tile([1, CW], F32, tag="rec")
                            nc.vector.reciprocal(rec[:], sums[:])
                            bc = pp.tile([DK, CW], F32, tag="pp")
                            nc.tensor.matmul(
                                bc[:],
                                lhsT=ones_sb[:],
                                rhs=rec[:],
                                start=True,
                                stop=True,
                            )
                            scale = small.tile([DK, CW], F32, tag="scale")
                            nc.vector.tensor_copy(scale[:], bc[:])
                            nc.vector.tensor_tensor(
                                yt[hl * DK : (hl + 1) * DK, :],
                                ops[hl][0:DK, :],
                                scale[:],
                                mybir.AluOpType.mult,
                            )
                    # output projection for this 512-token column
                    tcol = b * NJ + j
                    for mo in range(MO):
                        op_ps = pp.tile([P, CW], F32, tag="pp")
                        nc.tensor.matmul(
                            op_ps[:],
                            lhsT=wo_sb[:, mo * P : (mo + 1) * P],
                            rhs=yt[:],
                            start=True,
                            stop=True,
                        )
                        ob = obp.tile([P, CW], BF16, tag="ob")
                        nc.any.tensor_copy(ob[:], op_ps[:])
                        nc.sync.dma_start(
                            out_r[:, mo, tcol * CW : (tcol + 1) * CW], ob[:]
                        )
    nc.compile()
    return nc


def _get_module(plan, n_pat):
    key = (plan, n_pat)
    if key not in _BUILD_CACHE:
        _BUILD_CACHE[key] = _build(plan, n_pat)
    return _BUILD_CACHE[key]


def _prep_inputs(query, key, value, mask, W_q, b_q, W_k, b_k, W_v, b_v, W_o, b_o):
    def xt_of(x):
        x2 = np.asarray(x, np.float32).reshape(TOK, D)
        return np.ascontiguousarray(x2.T).astype(NPBF16)

    xq, xk, xv = xt_of(query), xt_of(key), xt_of(value)
    plan, pat_arr = _analyze_mask(mask)
    mpat = np.ascontiguousarray(pat_arr).astype(NPBF16)

    W_q = np.asarray(W_q, np.float32)
    W_k = np.asarray(W_k, np.float32)
    W_v = np.asarray(W_v, np.float32)
    W_o = np.asarray(W_o, np.float32)

    in_maps = []
    for c in range(N_CORES):
        cs = slice(c * CH, (c + 1) * CH)
        in_maps.append(
            {
                "xq": xq,
                "xk": xk,
                "xv": xv,
                "wq": np.ascontiguousarray(W_q[cs, :].T).astype(NPBF16),
                "wk": np.ascontiguousarray(W_k[cs, :].T).astype(NPBF16),
                "wv": np.ascontiguousarray(W_v[cs, :].T).astype(NPBF16),
                "wo": np.ascontiguousarray(W_o[:, cs].T).astype(NPBF16),
                "bq": np.asarray(b_q, np.float32)[cs].reshape(CH, 1).copy(),
                "bk": np.asarray(b_k, np.float32)[cs].reshape(CH, 1).copy(),
                "bv": np.asarray(b_v, np.float32)[cs].reshape(CH, 1).copy(),
                "mpat": mpat,
            }
        )
    return plan, mpat.shape[1], in_maps


def run(inputs, trace=False, trace_cores=None):
    """Build (cached), run on 8 cores, return (final_output, BassKernelResults)."""
    plan, n_pat, in_maps = _prep_inputs(**inputs)
    nc = _get_module(plan, n_pat)
    res = bass_utils.run_bass_kernel_spmd(
        nc,
        in_maps,
        core_ids=list(range(N_CORES)),
        trace=trace,
        trace_cores=trace_cores,
    )
    acc = np.zeros((D, TOK), np.float32)
    for c in range(N_CORES):
        acc += res.results[c]["out"].astype(np.float32)
    final = acc.T + np.asarray(inputs["b_o"], np.float32)[None, :]
    return final.reshape(B, S, D), res


def kernel(**inputs):
    return run(inputs, trace=False)[0]
